# revision 51
# baseline (speedup 1.0000x reference)
"""Trainium2 Bass kernel for nn_AtomEmbedding (embedding_lookup, memory-bound).

Strategy (pure data parallel over 8 NeuronCores):
  - All 30 integer feature columns become 66 indicator rows (45 one-hot class
    rows + 21 binary rows), precomputed host-side as exact 0/1 fp8 values.
  - Per 1024-atom pair: one fp8 DoubleRow matmul consumes the 66 indicator
    rows for BOTH 512-atom groups (the two DR k-subtiles carry group A/B with
    block-structured weights), then one bf16 matmul over the 96 2-packed bond
    rows accumulates into the same PSUM bank. Matmuls are grouped in 4-runs
    per weight set to minimize PE weight switching.
  - The 48 bond features are int8-quantized host-side (symmetric global
    scale, scale folded into the bf16 weights) and upconverted int8->bf16 in
    flight by the gpsimd software-DGE casting DMA - bond HBM reads are halved.
  - DMA rings (strict per-queue FIFO, priority q0>q1>q10, ~100-130GB/s each):
    gpsimd q0 = bond cast-DMAs; sync q1 = cat + half the lower-output DMAs
    (deferred one superblock so their drain-waits never head-of-line block
    the cat prefetch); scalar q10 = weights + the other output DMAs. Queue
    totals ~17-18MB each.
  - PSUM->SBUF drains run on ACT and DVE alternately in 2-bank units with the
    f32 bias fused (activation bias / tensor_scalar add).
  - Table edge semantics (element LUT default, ringsize unknown->6, ring-col
    constness) fold into weights via the delta trick + bias vector.
  - Output columns are permuted so the 4 constant ring cols sit at device rows
    60:64/124:128 and never leave the chip (120 of 128 rows DMA'd).
  - DRAM blobs are partition-major (16KB descriptors on the 8KB/chunk-row cat
    blob). Measured notes: plain (non-DR) 124/104-row matmul variants bench
    faster standalone but consistently lose ~50us in context; chunk-major
    layouts and 3-way output splits also regressed - keep this exact shape.
"""

import os
import sys

sys.path.insert(0, "/opt/trn_rl_repo")
os.environ.setdefault("MYCRO_LOCAL_CACHE", "1")

import ml_dtypes
import numpy as np

import concourse.bacc as bacc
import concourse.bass as bass
import concourse.mybir as mybir
import concourse.tile as tile
from concourse.bass_utils import run_bass_kernel_spmd

F32 = mybir.dt.float32
BF16 = mybir.dt.bfloat16
FP8 = mybir.dt.float8e4
I8 = mybir.dt.int8
NPBF16 = ml_dtypes.bfloat16
NPFP8 = ml_dtypes.float8_e4m3fn

N_CORES = 8
N_TOTAL = 1_500_000
N_SHARD = N_TOTAL // N_CORES  # 187500
G = 512                       # atoms per group (one matmul output column half)
PAIR = 2 * G                  # atoms per pair (2 groups via DoubleRow subtiles)
PAIRS_PER_CHUNK = 8
N_CHUNKS = 23
N_PAIRS = N_CHUNKS * PAIRS_PER_CHUNK  # 184
NPAD = N_PAIRS * PAIR         # 188416 padded atoms per core
FREE = PAIRS_PER_CHUNK * G    # 4096 output columns per chunk

NCAT = 66                     # indicator rows per group (45 one-hot + 21 bin)
NBOND = 48
NBB = NBOND                   # bond matmul rows per group
NB2 = 2 * NBB                 # bond-side rows 2-packed on partitions
NOUT = 64
NKEEP = 60                    # output cols per group shipped to HBM
# device output column permutation: ring block (cols 8:12, constant) goes last
PERM64 = list(range(0, 8)) + list(range(12, 64)) + list(range(8, 12))

BOND_MODE = os.environ.get("BOND_MODE", "dmacast")  # "bf16" | "dmacast"


def build_tables(inputs):
    """Fold all embedding tables + linear weights into device constants."""
    g = {k: np.asarray(v, dtype=np.float64) if np.asarray(v).dtype.kind == "f"
         else np.asarray(v) for k, v in inputs.items()}
    elut = g["element_lut"].astype(np.int64)
    rvals = g["ring_values"].astype(np.int64)
    ft = g["func_tables"]
    frw = g["func_reduce_w"]

    def func_delta(j):
        Rj = frw[:, 2 * j:2 * j + 2]
        return (ft[j, 1] - ft[j, 0]) @ Rj.T

    rows = []  # (source col, compare value, weight row [64])

    def add(col, v, c0, w):
        wr = np.zeros(NOUT)
        wr[c0:c0 + len(w)] = w
        rows.append((col, float(v), wr))

    e_def = int(np.clip(elut[0], 0, 6))
    for v in range(1, 17):
        idx = int(np.clip(elut[v], 0, 6))
        if idx != e_def:
            add(0, v, 0, g["element_embed"][idx] - g["element_embed"][e_def])
    for k in range(1, 7):
        add(1, k, 4, g["degree_embed"][k] - g["degree_embed"][0])
    for k in range(1, 8):
        add(2, k, 12, g["charge_embed"][k] - g["charge_embed"][0])
    for k in range(1, 6):
        add(3, k, 20, g["hybrid_embed"][k] - g["hybrid_embed"][0])
    for k in range(1, 5):
        add(6, k, 24, g["hydrogen_embed"][k] - g["hydrogen_embed"][0])
    seen = set()
    for i in range(7):
        v = int(rvals[i])
        if v in seen:
            continue
        seen.add(v)
        w = g["ringsize_embed"][i] - g["ringsize_embed"][6]
        if np.any(w != 0.0):
            add(27, v, 36, w)
    for k in range(1, 5):
        add(28, k, 40, g["aroma_num_embed"][k] - g["aroma_num_embed"][0])
    for k in range(1, 8):
        add(29, k, 44, g["fused_if_embed"][k] - g["fused_if_embed"][0])
    assert len(rows) == 45, len(rows)
    # binary indicator rows (value==1 compare is the identity); the last 4
    # func flags ride the bond-side matmul instead (balances the 2-packed
    # partition budgets: cat 2*62=124, bond 2*52=104)
    add(4, 1, 16, g["aromatic_embed"][1] - g["aromatic_embed"][0])
    add(25, 1, 32, g["h_don_embed"][1] - g["h_don_embed"][0])
    add(26, 1, 34, g["h_acc_embed"][1] - g["h_acc_embed"][0])
    for j in range(18):
        add(7 + j, 1, 28, func_delta(j))
    assert len(rows) == NCAT

    cat_cols = np.array([c for c, _, _ in rows])
    cat_vals = np.array([v for _, v, _ in rows], dtype=np.float32)
    bin_cols = np.array([], dtype=np.int64)
    W_cat = np.stack([w for _, _, w in rows])[:, PERM64]   # [66, 64]

    bias = np.zeros(NOUT)
    bias[0:4] = g["element_embed"][e_def]
    bias[4:8] = g["degree_embed"][0]
    bias[8:12] = g["ring_embed"][1]   # ring col: clip(ring+1,0,1)==1 always
    bias[12:16] = g["charge_embed"][0]
    bias[16:20] = g["aromatic_embed"][0]
    bias[20:24] = g["hybrid_embed"][0]
    bias[24:28] = g["hydrogen_embed"][0]
    bias[28:32] = g["func_reduce_b"] + sum(
        ft[j, 0] @ frw[:, 2 * j:2 * j + 2].T for j in range(18))
    bias[32:34] = g["h_don_embed"][0]
    bias[34:36] = g["h_acc_embed"][0]
    bias[36:40] = g["ringsize_embed"][6]
    bias[40:44] = g["aroma_num_embed"][0]
    bias[44:48] = g["fused_if_embed"][0]
    bias[48:64] = g["bond_env_b"]
    bias_p = bias[PERM64]

    # DoubleRow cat weights [NCAT, 2, 128]: subtile 0 -> out rows 0:64
    # (group A), subtile 1 -> rows 64:128 (group B)
    w_cat = np.zeros((NCAT, 2, 128), NPFP8)
    w_cat[:, 0, 0:64] = W_cat
    w_cat[:, 1, 64:128] = W_cat

    # bond weights (quant scale folded), block-diagonal over the 2-pack
    s_x = float(np.abs(np.asarray(inputs["atom_inputs"])[:, 30:]).max()) / 127.0
    Wb = np.zeros((NBB, NOUT))
    Wb[0:NBOND, 48:64] = g["bond_env_w"].T * s_x
    Wb = Wb[:, PERM64]
    w_bnd = np.zeros((NB2, 128), NPBF16)
    w_bnd[0:NBB, 0:64] = Wb
    w_bnd[NBB:, 64:128] = Wb

    bias2 = np.tile(bias_p, 2).reshape(128, 1).astype(np.float32)
    consts = {"w_cat": np.ascontiguousarray(w_cat.reshape(NCAT, 256)),
              "w_bnd": np.ascontiguousarray(w_bnd), "bias": bias2}
    ring_fill = g["ring_embed"][1].astype(np.float32)
    return consts, cat_cols, cat_vals, bin_cols, s_x, ring_fill


def build_nc(bond_mode=BOND_MODE):
    nc = bacc.Bacc(None)
    cat_d = nc.dram_tensor("cat", [NCAT, N_CHUNKS, 2 * FREE], FP8,
                           kind="ExternalInput")
    bnd8_d = nc.dram_tensor("bnd8", [NB2, N_CHUNKS, FREE], I8,
                            kind="ExternalInput")
    wcat_d = nc.dram_tensor("w_cat", [NCAT, 256], FP8,
                            kind="ExternalInput")
    wbnd_d = nc.dram_tensor("w_bnd", [NB2, 128], BF16, kind="ExternalInput")
    bias_d = nc.dram_tensor("bias", [128, 1], F32, kind="ExternalInput")
    out_d = nc.dram_tensor("out", [2 * NKEEP, N_CHUNKS, FREE], BF16,
                           kind="ExternalOutput")

    # DMA ring plan (strict per-queue FIFO; queue priority q0 > q1 > q10;
    # ~100-130 GB/s per queue):
    #   gpsimd(q0): all bond chunks, int8 HBM -> bf16 SBUF casting software
    #               DGE (halves bond HBM reads)
    #   sync  (q1): cat in-stream, then outputs of late superblocks (their
    #               data follows cat in FIFO order, which is fine: by then
    #               cat is fully transferred)
    #   scalar(q10): weights at t=0, outputs of early/mid superblocks
    with tile.TileContext(nc) as tc:
        with (
            tc.tile_pool(name="consts", bufs=1) as cpool,
            tc.tile_pool(name="cat", bufs=4) as catp,
            tc.tile_pool(name="bnd", bufs=4) as bndp,
            tc.tile_pool(name="outs", bufs=4) as outp,
            tc.tile_pool(name="pso", bufs=4, space="PSUM") as pso,
        ):
            wcat_t = cpool.tile([NCAT, 2, 128], FP8)
            nc.scalar.dma_start(wcat_t[:], wcat_d[:])
            wbnd_t = cpool.tile([NB2, 128], BF16)
            nc.scalar.dma_start(wbnd_t[:], wbnd_d[:])
            bias_t = cpool.tile([128, 1], F32)
            nc.scalar.dma_start(bias_t[:], bias_d[:])

            # small first superblocks so compute starts sooner; small last so
            # the final out-DMA tail is short
            spans = [(0, 1), (1, 1)]
            c = 2
            while c < N_CHUNKS:
                s = min(2, N_CHUNKS - c)
                spans.append((c, s))
                c += s
            drain_i = 0
            # out-DMAs whose trigger must ride the sync engine are issued two
            # superblocks late so their drain-waits are already satisfied when
            # sync reaches them (keeps cat prefetch ~3 superblocks deep)
            deferred = []
            for bi, (c, span) in enumerate(spans):
                cat_t = catp.tile([NCAT, span, 2, PAIRS_PER_CHUNK, G], FP8,
                                  tag="cat")
                bnd_t = bndp.tile([NB2, span, PAIRS_PER_CHUNK, G], BF16,
                                  tag="bnd")
                # first superblock's cat rides q0 ahead of the bond
                # prefetch flood (strict queue priority would otherwise
                # starve the q1 cat transfer for ~20us at startup)
                ri = nc.gpsimd if bi == 0 else nc.sync
                ri.dma_start(cat_t[:], cat_d[:, c:c + span])
                nc.gpsimd.dma_start(bnd_t[:], bnd8_d[:, c:c + span])
                out_t = outp.tile([128, span, FREE], BF16, tag="out")
                for j in range(span):
                    for half in range(2):
                        # 4 pairs per half-chunk: 4 fp8 cat matmuls
                        # back-to-back, then 4 bf16 bond matmuls (minimizes
                        # weight switching); drains in 2-bank units on
                        # ACT/DVE alternately with the f32 bias fused
                        psA = pso.tile([128, 2 * G], F32, tag="ps")
                        psB = pso.tile([128, 2 * G], F32, tag="ps")
                        halves = [psA[:, 0:G], psA[:, G:2 * G],
                                  psB[:, 0:G], psB[:, G:2 * G]]
                        p0 = 4 * half
                        for k in range(4):
                            nc.tensor.matmul(halves[k], wcat_t[:],
                                             cat_t[:, j, :, p0 + k, :],
                                             start=True, stop=False,
                                             perf_mode=mybir.MatmulPerfMode
                                             .DoubleRow)
                        for k in range(4):
                            nc.tensor.matmul(halves[k], wbnd_t[:],
                                             bnd_t[:, j, p0 + k, :],
                                             start=False, stop=True)
                        slA = bass.ts(2 * half, 2 * G)
                        slB = bass.ts(2 * half + 1, 2 * G)
                        if drain_i % 2 == 0:
                            nc.scalar.activation(
                                out_t[:, j, slA], psA[:],
                                mybir.ActivationFunctionType.Identity,
                                bias=bias_t[:], scale=1.0)
                            nc.vector.tensor_scalar(
                                out_t[:, j, slB], psB[:], bias_t[:], None,
                                mybir.AluOpType.add)
                        else:
                            nc.vector.tensor_scalar(
                                out_t[:, j, slA], psA[:], bias_t[:], None,
                                mybir.AluOpType.add)
                            nc.scalar.activation(
                                out_t[:, j, slB], psB[:],
                                mybir.ActivationFunctionType.Identity,
                                bias=bias_t[:], scale=1.0)
                        drain_i += 1
                # outputs: rows 0:60 always on scalar (q10); rows 64:124
                # alternate scalar / sync-deferred to balance queue bytes
                nc.scalar.dma_start(out_d[0:NKEEP, c:c + span],
                                    out_t[0:NKEEP])
                argsB = (out_d[NKEEP:2 * NKEEP, c:c + span],
                         out_t[64:64 + NKEEP])
                if bi % 2 == 0:
                    nc.scalar.dma_start(*argsB)
                else:
                    deferred.append(argsB)
                    if len(deferred) > 1:
                        nc.sync.dma_start(*deferred.pop(0))
            for args in deferred:
                nc.sync.dma_start(*args)
    nc.compile()
    return nc


def shard_blobs(ai, core, cat_cols, cat_vals, bin_cols, s_x,
                bond_mode=BOND_MODE):
    """Slice one core's shard into the device blobs (partition-major)."""
    shard = ai[core * N_SHARD:(core + 1) * N_SHARD]
    padded = np.zeros((NPAD, ai.shape[1]), np.float32)
    padded[:N_SHARD] = shard
    # [chunk, pair, group, atom, col]
    v = padded.reshape(N_CHUNKS, PAIRS_PER_CHUNK, 2, G, ai.shape[1])
    oh = (v[..., cat_cols] == cat_vals).astype(np.uint8)  # [c,p,s,a,66]
    # DoubleRow ifmap layout: [r, c, (s, p, a)]
    cat = (oh * np.uint8(0x38)).transpose(4, 0, 2, 1, 3) \
        .reshape(NCAT, N_CHUNKS, 2 * FREE)
    cat = np.ascontiguousarray(cat).view(NPFP8)
    q = np.clip(np.round(v[..., 30:] * (1.0 / s_x)), -127, 127)  # [c,p,s,a,48]
    bnd = q.transpose(2, 4, 0, 1, 3).reshape(NB2, N_CHUNKS, FREE)
    bnd8 = np.ascontiguousarray(bnd).astype(np.int8)
    return cat, bnd8


def unshard_out(o, ring_fill):
    """[120, N_CHUNKS, FREE] device layout -> [NPAD, 64] atom-major."""
    # rows = (group s, kept col j); cols = (chunk, pair, atom)
    t = np.asarray(o, dtype=np.float32).reshape(2, NKEEP, N_CHUNKS,
                                                PAIRS_PER_CHUNK, G)
    t = t.transpose(2, 3, 0, 4, 1).reshape(NPAD, NKEEP)  # [c,p,s,a,j]
    full = np.empty((NPAD, NOUT), np.float32)
    full[:, PERM64[:NKEEP]] = t
    full[:, 8:12] = ring_fill
    return full


def _install_ntff_hook():
    """Register the axon NTFF profile hook that this image's antenv lacks."""
    import types
    try:
        import antenv.axon_hooks  # noqa: F401
        return
    except ImportError:
        pass
    try:
        from trn_agent_boot.trn_boot import _ntff_profile_via_ctypes
        hook = _ntff_profile_via_ctypes("/opt/axon/libaxon_pjrt.so")
        mod = types.ModuleType("antenv.axon_hooks")
        _state = {"hook": hook}
        mod.set_axon_ntff_profile_hook = lambda h: _state.__setitem__("hook", h)
        mod.get_axon_ntff_profile_hook = lambda: _state["hook"]
        sys.modules["antenv.axon_hooks"] = mod
        import antenv
        antenv.axon_hooks = mod
    except Exception as e:  # profiling is best-effort
        print(f"ntff hook install failed: {e}", file=sys.stderr)


def kernel(**inputs):
    consts, cat_cols, cat_vals, bin_cols, s_x, ring_fill = build_tables(inputs)
    ai = np.ascontiguousarray(np.asarray(inputs["atom_inputs"], dtype=np.float32))
    assert ai.shape == (N_TOTAL, 78), ai.shape

    in_maps = []
    for i in range(N_CORES):
        cat, bnd8 = shard_blobs(ai, i, cat_cols, cat_vals, bin_cols, s_x)
        in_maps.append({"cat": cat, "bnd8": bnd8, **consts})

    trace = bool(int(os.environ.get("KERNEL_TRACE", "0")))
    if trace:
        _install_ntff_hook()
    nc = build_nc()
    res = run_bass_kernel_spmd(
        nc, in_maps, core_ids=list(range(N_CORES)), trace=trace,
    )
    kernel.last_result = res

    outs = []
    for i in range(N_CORES):
        outs.append(unshard_out(res.results[i]["out"], ring_fill)[:N_SHARD])
    return np.ascontiguousarray(np.concatenate(outs, axis=0))


kernel.last_result = None


# revision 52
# speedup vs baseline: 1.1287x; 1.1287x over previous
"""Trainium2 Bass kernel for nn_AtomEmbedding (embedding_lookup, memory-bound).

Strategy (pure data parallel over 8 NeuronCores):
  - All 30 integer feature columns become 66 indicator rows (45 one-hot class
    rows + 21 binary rows), precomputed host-side as exact 0/1 fp8 values.
  - Per 1024-atom pair: one fp8 DoubleRow matmul consumes the 66 indicator
    rows for BOTH 512-atom groups (the two DR k-subtiles carry group A/B with
    block-structured weights), then one bf16 matmul over the 96 2-packed bond
    rows accumulates into the same PSUM bank. Matmuls are grouped in 4-runs
    per weight set to minimize PE weight switching.
  - The 48 bond features are int8-quantized host-side (symmetric global
    scale, scale folded into the bf16 weights) and upconverted int8->bf16 in
    flight by the gpsimd software-DGE casting DMA - bond HBM reads are halved.
  - DMA rings (strict per-queue FIFO, priority q0>q1>q10, ~100-130GB/s each):
    gpsimd q0 = bond cast-DMAs; sync q1 = cat + half the lower-output DMAs
    (deferred one superblock so their drain-waits never head-of-line block
    the cat prefetch); scalar q10 = weights + the other output DMAs. Queue
    totals ~17-18MB each.
  - PSUM->SBUF drains run on ACT and DVE alternately in 2-bank units with the
    f32 bias fused (activation bias / tensor_scalar add).
  - Table edge semantics (element LUT default, ringsize unknown->6, ring-col
    constness) fold into weights via the delta trick + bias vector.
  - Output columns are permuted so the 4 constant ring cols sit at device rows
    60:64/124:128 and never leave the chip (120 of 128 rows DMA'd).
  - DRAM blobs are partition-major (16KB descriptors on the 8KB/chunk-row cat
    blob). Measured notes: plain (non-DR) 124/104-row matmul variants bench
    faster standalone but consistently lose ~50us in context; chunk-major
    layouts and 3-way output splits also regressed - keep this exact shape.
"""

import os
import sys

sys.path.insert(0, "/opt/trn_rl_repo")
os.environ.setdefault("MYCRO_LOCAL_CACHE", "1")

import ml_dtypes
import numpy as np

import concourse.bacc as bacc
import concourse.bass as bass
import concourse.mybir as mybir
import concourse.tile as tile
from concourse.bass_utils import run_bass_kernel_spmd

F32 = mybir.dt.float32
BF16 = mybir.dt.bfloat16
FP8 = mybir.dt.float8e4
I8 = mybir.dt.int8
NPBF16 = ml_dtypes.bfloat16
NPFP8 = ml_dtypes.float8_e4m3fn

N_CORES = 8
N_TOTAL = 1_500_000
N_SHARD = N_TOTAL // N_CORES  # 187500
G = 512                       # atoms per group (one matmul output column half)
PAIR = 2 * G                  # atoms per pair (2 groups via DoubleRow subtiles)
PAIRS_PER_CHUNK = 8
N_CHUNKS = 23
N_PAIRS = N_CHUNKS * PAIRS_PER_CHUNK  # 184
NPAD = N_PAIRS * PAIR         # 188416 padded atoms per core
FREE = PAIRS_PER_CHUNK * G    # 4096 output columns per chunk

NCAT = 66                     # indicator rows per group (45 one-hot + 21 bin)
NBOND = 48
NBB = NBOND                   # bond matmul rows per group
NB2 = 2 * NBB                 # bond-side rows 2-packed on partitions
NOUT = 64
NKEEP = 60                    # output cols per group shipped to HBM
# device output column permutation: ring block (cols 8:12, constant) goes last
PERM64 = list(range(0, 8)) + list(range(12, 64)) + list(range(8, 12))

BOND_MODE = os.environ.get("BOND_MODE", "dmacast")  # "bf16" | "dmacast"


def build_tables(inputs):
    """Fold all embedding tables + linear weights into device constants."""
    g = {k: np.asarray(v, dtype=np.float64) if np.asarray(v).dtype.kind == "f"
         else np.asarray(v) for k, v in inputs.items()}
    elut = g["element_lut"].astype(np.int64)
    rvals = g["ring_values"].astype(np.int64)
    ft = g["func_tables"]
    frw = g["func_reduce_w"]

    def func_delta(j):
        Rj = frw[:, 2 * j:2 * j + 2]
        return (ft[j, 1] - ft[j, 0]) @ Rj.T

    rows = []  # (source col, compare value, weight row [64])

    def add(col, v, c0, w):
        wr = np.zeros(NOUT)
        wr[c0:c0 + len(w)] = w
        rows.append((col, float(v), wr))

    e_def = int(np.clip(elut[0], 0, 6))
    for v in range(1, 17):
        idx = int(np.clip(elut[v], 0, 6))
        if idx != e_def:
            add(0, v, 0, g["element_embed"][idx] - g["element_embed"][e_def])
    for k in range(1, 7):
        add(1, k, 4, g["degree_embed"][k] - g["degree_embed"][0])
    for k in range(1, 8):
        add(2, k, 12, g["charge_embed"][k] - g["charge_embed"][0])
    for k in range(1, 6):
        add(3, k, 20, g["hybrid_embed"][k] - g["hybrid_embed"][0])
    for k in range(1, 5):
        add(6, k, 24, g["hydrogen_embed"][k] - g["hydrogen_embed"][0])
    seen = set()
    for i in range(7):
        v = int(rvals[i])
        if v in seen:
            continue
        seen.add(v)
        w = g["ringsize_embed"][i] - g["ringsize_embed"][6]
        if np.any(w != 0.0):
            add(27, v, 36, w)
    for k in range(1, 5):
        add(28, k, 40, g["aroma_num_embed"][k] - g["aroma_num_embed"][0])
    for k in range(1, 8):
        add(29, k, 44, g["fused_if_embed"][k] - g["fused_if_embed"][0])
    assert len(rows) == 45, len(rows)
    # binary indicator rows (value==1 compare is the identity); the last 4
    # func flags ride the bond-side matmul instead (balances the 2-packed
    # partition budgets: cat 2*62=124, bond 2*52=104)
    add(4, 1, 16, g["aromatic_embed"][1] - g["aromatic_embed"][0])
    add(25, 1, 32, g["h_don_embed"][1] - g["h_don_embed"][0])
    add(26, 1, 34, g["h_acc_embed"][1] - g["h_acc_embed"][0])
    for j in range(18):
        add(7 + j, 1, 28, func_delta(j))
    assert len(rows) == NCAT

    cat_cols = np.array([c for c, _, _ in rows])
    cat_vals = np.array([v for _, v, _ in rows], dtype=np.float32)
    bin_cols = np.array([], dtype=np.int64)
    W_cat = np.stack([w for _, _, w in rows])[:, PERM64]   # [66, 64]

    bias = np.zeros(NOUT)
    bias[0:4] = g["element_embed"][e_def]
    bias[4:8] = g["degree_embed"][0]
    bias[8:12] = g["ring_embed"][1]   # ring col: clip(ring+1,0,1)==1 always
    bias[12:16] = g["charge_embed"][0]
    bias[16:20] = g["aromatic_embed"][0]
    bias[20:24] = g["hybrid_embed"][0]
    bias[24:28] = g["hydrogen_embed"][0]
    bias[28:32] = g["func_reduce_b"] + sum(
        ft[j, 0] @ frw[:, 2 * j:2 * j + 2].T for j in range(18))
    bias[32:34] = g["h_don_embed"][0]
    bias[34:36] = g["h_acc_embed"][0]
    bias[36:40] = g["ringsize_embed"][6]
    bias[40:44] = g["aroma_num_embed"][0]
    bias[44:48] = g["fused_if_embed"][0]
    bias[48:64] = g["bond_env_b"]
    bias_p = bias[PERM64]

    # DoubleRow cat weights [NCAT, 2, 128]: subtile 0 -> out rows 0:64
    # (group A), subtile 1 -> rows 64:128 (group B)
    w_cat = np.zeros((NCAT, 2, 128), NPFP8)
    w_cat[:, 0, 0:64] = W_cat
    w_cat[:, 1, 64:128] = W_cat

    # bond weights (quant scale folded), block-diagonal over the 2-pack
    s_x = float(np.abs(np.asarray(inputs["atom_inputs"])[:, 30:]).max()) / 127.0
    Wb = np.zeros((NBB, NOUT))
    Wb[0:NBOND, 48:64] = g["bond_env_w"].T * s_x
    Wb = Wb[:, PERM64]
    w_bnd = np.zeros((NB2, 128), NPBF16)
    w_bnd[0:NBB, 0:64] = Wb
    w_bnd[NBB:, 64:128] = Wb

    bias2 = np.tile(bias_p, 2).reshape(128, 1).astype(np.float32)
    consts = {"w_cat": np.ascontiguousarray(w_cat.reshape(NCAT, 256)),
              "w_bnd": np.ascontiguousarray(w_bnd), "bias": bias2}
    ring_fill = g["ring_embed"][1].astype(np.float32)
    return consts, cat_cols, cat_vals, bin_cols, s_x, ring_fill


def build_nc(bond_mode=BOND_MODE):
    nc = bacc.Bacc(None)
    cat_d = nc.dram_tensor("cat", [NCAT, N_CHUNKS, 2 * FREE], FP8,
                           kind="ExternalInput")
    bnd8_d = nc.dram_tensor("bnd8", [NB2, N_CHUNKS, FREE], I8,
                            kind="ExternalInput")
    wcat_d = nc.dram_tensor("w_cat", [NCAT, 256], FP8,
                            kind="ExternalInput")
    wbnd_d = nc.dram_tensor("w_bnd", [NB2, 128], BF16, kind="ExternalInput")
    bias_d = nc.dram_tensor("bias", [128, 1], F32, kind="ExternalInput")
    out_d = nc.dram_tensor("out", [2 * NKEEP, N_CHUNKS, FREE], BF16,
                           kind="ExternalOutput")

    # DMA ring plan (strict per-queue FIFO; queue priority q0 > q1 > q10;
    # ~100-130 GB/s per queue):
    #   gpsimd(q0): all bond chunks, int8 HBM -> bf16 SBUF casting software
    #               DGE (halves bond HBM reads)
    #   sync  (q1): cat in-stream, then outputs of late superblocks (their
    #               data follows cat in FIFO order, which is fine: by then
    #               cat is fully transferred)
    #   scalar(q10): weights at t=0, outputs of early/mid superblocks
    with tile.TileContext(nc) as tc:
        with (
            tc.tile_pool(name="consts", bufs=1) as cpool,
            tc.tile_pool(name="cat", bufs=4) as catp,
            tc.tile_pool(name="bnd", bufs=4) as bndp,
            tc.tile_pool(name="outs", bufs=4) as outp,
            tc.tile_pool(name="pso", bufs=4, space="PSUM") as pso,
        ):
            wcat_t = cpool.tile([NCAT, 2, 128], FP8)
            nc.scalar.dma_start(wcat_t[:], wcat_d[:])
            wbnd_t = cpool.tile([NB2, 128], BF16)
            nc.scalar.dma_start(wbnd_t[:], wbnd_d[:])
            bias_t = cpool.tile([128, 1], F32)
            nc.scalar.dma_start(bias_t[:], bias_d[:])

            # small first superblocks so compute starts sooner; small last so
            # the final out-DMA tail is short
            spans = [(0, 1), (1, 1)]
            c = 2
            while c < N_CHUNKS:
                s = min(2, N_CHUNKS - c)
                spans.append((c, s))
                c += s
            drain_i = 0
            # out-DMAs whose trigger must ride the sync engine are issued two
            # superblocks late so their drain-waits are already satisfied when
            # sync reaches them (keeps cat prefetch ~3 superblocks deep)
            deferred = []
            for bi, (c, span) in enumerate(spans):
                cat_t = catp.tile([NCAT, span, 2, PAIRS_PER_CHUNK, G], FP8,
                                  tag="cat")
                bnd_t = bndp.tile([NB2, span, PAIRS_PER_CHUNK, G], BF16,
                                  tag="bnd")
                nc.sync.dma_start(cat_t[:], cat_d[:, c:c + span])
                nc.gpsimd.dma_start(bnd_t[:], bnd8_d[:, c:c + span])
                out_t = outp.tile([128, span, FREE], BF16, tag="out")
                for j in range(span):
                    for half in range(2):
                        # 4 pairs per half-chunk: 4 fp8 cat matmuls
                        # back-to-back, then 4 bf16 bond matmuls (minimizes
                        # weight switching); drains in 2-bank units on
                        # ACT/DVE alternately with the f32 bias fused
                        psA = pso.tile([128, 2 * G], F32, tag="ps")
                        psB = pso.tile([128, 2 * G], F32, tag="ps")
                        halves = [psA[:, 0:G], psA[:, G:2 * G],
                                  psB[:, 0:G], psB[:, G:2 * G]]
                        p0 = 4 * half
                        for k in range(4):
                            nc.tensor.matmul(halves[k], wcat_t[:],
                                             cat_t[:, j, :, p0 + k, :],
                                             start=True, stop=False,
                                             perf_mode=mybir.MatmulPerfMode
                                             .DoubleRow)
                        for k in range(4):
                            nc.tensor.matmul(halves[k], wbnd_t[:],
                                             bnd_t[:, j, p0 + k, :],
                                             start=False, stop=True)
                        slA = bass.ts(2 * half, 2 * G)
                        slB = bass.ts(2 * half + 1, 2 * G)
                        if drain_i % 2 == 0:
                            nc.scalar.activation(
                                out_t[:, j, slA], psA[:],
                                mybir.ActivationFunctionType.Identity,
                                bias=bias_t[:], scale=1.0)
                            nc.vector.tensor_scalar(
                                out_t[:, j, slB], psB[:], bias_t[:], None,
                                mybir.AluOpType.add)
                        else:
                            nc.vector.tensor_scalar(
                                out_t[:, j, slA], psA[:], bias_t[:], None,
                                mybir.AluOpType.add)
                            nc.scalar.activation(
                                out_t[:, j, slB], psB[:],
                                mybir.ActivationFunctionType.Identity,
                                bias=bias_t[:], scale=1.0)
                        drain_i += 1
                # outputs: rows 0:60 always on scalar (q10); rows 64:124
                # alternate scalar / sync-deferred to balance queue bytes
                nc.scalar.dma_start(out_d[0:NKEEP, c:c + span],
                                    out_t[0:NKEEP])
                argsB = (out_d[NKEEP:2 * NKEEP, c:c + span],
                         out_t[64:64 + NKEEP])
                if bi % 2 == 0:
                    nc.scalar.dma_start(*argsB)
                else:
                    deferred.append(argsB)
                    if len(deferred) > 1:
                        nc.sync.dma_start(*deferred.pop(0))
            for args in deferred:
                nc.sync.dma_start(*args)
    nc.compile()
    return nc


def shard_blobs(ai, core, cat_cols, cat_vals, bin_cols, s_x,
                bond_mode=BOND_MODE):
    """Slice one core's shard into the device blobs (partition-major)."""
    shard = ai[core * N_SHARD:(core + 1) * N_SHARD]
    padded = np.zeros((NPAD, ai.shape[1]), np.float32)
    padded[:N_SHARD] = shard
    # [chunk, pair, group, atom, col]
    v = padded.reshape(N_CHUNKS, PAIRS_PER_CHUNK, 2, G, ai.shape[1])
    oh = (v[..., cat_cols] == cat_vals).astype(np.uint8)  # [c,p,s,a,66]
    # DoubleRow ifmap layout: [r, c, (s, p, a)]
    cat = (oh * np.uint8(0x38)).transpose(4, 0, 2, 1, 3) \
        .reshape(NCAT, N_CHUNKS, 2 * FREE)
    cat = np.ascontiguousarray(cat).view(NPFP8)
    q = np.clip(np.round(v[..., 30:] * (1.0 / s_x)), -127, 127)  # [c,p,s,a,48]
    bnd = q.transpose(2, 4, 0, 1, 3).reshape(NB2, N_CHUNKS, FREE)
    bnd8 = np.ascontiguousarray(bnd).astype(np.int8)
    return cat, bnd8


def unshard_out(o, ring_fill):
    """[120, N_CHUNKS, FREE] device layout -> [NPAD, 64] atom-major."""
    # rows = (group s, kept col j); cols = (chunk, pair, atom)
    t = np.asarray(o, dtype=np.float32).reshape(2, NKEEP, N_CHUNKS,
                                                PAIRS_PER_CHUNK, G)
    t = t.transpose(2, 3, 0, 4, 1).reshape(NPAD, NKEEP)  # [c,p,s,a,j]
    full = np.empty((NPAD, NOUT), np.float32)
    full[:, PERM64[:NKEEP]] = t
    full[:, 8:12] = ring_fill
    return full


def _install_ntff_hook():
    """Register the axon NTFF profile hook that this image's antenv lacks."""
    import types
    try:
        import antenv.axon_hooks  # noqa: F401
        return
    except ImportError:
        pass
    try:
        from trn_agent_boot.trn_boot import _ntff_profile_via_ctypes
        hook = _ntff_profile_via_ctypes("/opt/axon/libaxon_pjrt.so")
        mod = types.ModuleType("antenv.axon_hooks")
        _state = {"hook": hook}
        mod.set_axon_ntff_profile_hook = lambda h: _state.__setitem__("hook", h)
        mod.get_axon_ntff_profile_hook = lambda: _state["hook"]
        sys.modules["antenv.axon_hooks"] = mod
        import antenv
        antenv.axon_hooks = mod
    except Exception as e:  # profiling is best-effort
        print(f"ntff hook install failed: {e}", file=sys.stderr)


def kernel(**inputs):
    consts, cat_cols, cat_vals, bin_cols, s_x, ring_fill = build_tables(inputs)
    ai = np.ascontiguousarray(np.asarray(inputs["atom_inputs"], dtype=np.float32))
    assert ai.shape == (N_TOTAL, 78), ai.shape

    in_maps = []
    for i in range(N_CORES):
        cat, bnd8 = shard_blobs(ai, i, cat_cols, cat_vals, bin_cols, s_x)
        in_maps.append({"cat": cat, "bnd8": bnd8, **consts})

    trace = bool(int(os.environ.get("KERNEL_TRACE", "0")))
    if trace:
        _install_ntff_hook()
    nc = build_nc()
    res = run_bass_kernel_spmd(
        nc, in_maps, core_ids=list(range(N_CORES)), trace=trace,
    )
    kernel.last_result = res

    outs = []
    for i in range(N_CORES):
        outs.append(unshard_out(res.results[i]["out"], ring_fill)[:N_SHARD])
    return np.ascontiguousarray(np.concatenate(outs, axis=0))


kernel.last_result = None
